# revision 55
# baseline (speedup 1.0000x reference)
"""Trainium2 Bass kernel for nn_Attention_8366596292664.

Dense transformer block: qkv proj -> RoPE -> GQA causal attention ->
out proj -> RMSNorm.  B=4, S=2048, H=2048, 16 heads (hd=128), 4 KV heads.

Sharding: 8 cores = (4 batches) x (2 interleaved query-row parities).
Core (b, par) computes the full block for query rows {par, par+2, ...} of
batch b.  Interleaving the query rows by parity makes the causal structure
identical on every core, so one SPMD program serves all 8 cores; the
parity enters only through the data (a 1-column roll of x^T, cos/sin
tables, and the output row scatter).

Structure (v2):
  Phase A: k/v projections.  x^T streams in as 16 per-h-tile DMAs while
    8 interleaved PSUM accumulation groups consume the tiles h-outer, so
    the PE starts ~3us into the kernel instead of waiting for the full
    8.4MB transfer.  Ends with q-proj of heads 0-1 to prime phase B.
  Phase B: per-head software pipeline [q-proj head h+2 | attention head h].
    Overlaps the PE-heavy projection with the ACT-heavy softmax.  The
    causal diagonal band is triangularized: ragged matmuls with 64-column
    offsets skip ~30% of score/AV/exp work; a single 64-col edge mask +
    memsets replace the old full mask tensor.  Edge masking and the
    denominator quad-reduction run on the otherwise idle GpSimd engine.
  Phase C: out-proj + RMSNorm.  w_proj is half-prefetched during phase B
    (no PE gap at the boundary); final scaling is one fused
    scalar_tensor_tensor (ot*rr)*norm_w per chunk.
"""

import numpy as np
import ml_dtypes

BF16 = ml_dtypes.bfloat16

# ---------------------------------------------------------------- config
P = 128          # partitions
HD = 128         # head dim
HH = HD // 2     # rope half
G = 4            # GQA group size

B = 4
S = 2048
H = 2048
N_CORES = 8

NH = H // HD          # 16 q heads
NKV = NH // G         # 4 kv heads
KVC = NKV * HD        # 512 kv columns
HT = H // P           # 16 h-tiles (contraction tiles)
S_LOC = S // 2        # 1024 local q rows per core
IT = 512              # i-tile (queries per score tile, = 1 psum bank fp32)
NT_I = S_LOC // IT    # 2 i-slots
SPAN = S // NT_I      # 1024 global rows per slot
JB = SPAN // P        # 8 j-tiles in the diagonal band of each slot
OT = 512              # output-proj column tile
NO = H // OT          # 4
WPA = 8               # wp h-tiles prefetched during phase B

RMS_EPS = 1e-6
SCALE = 1.0 / float(np.sqrt(np.float32(HD)))

# engine routing knobs
GP_EDGES = True       # edge-mask muls on gpsimd (else vector)
GP_QUADS = False      # denominator quad adds on gpsimd (else vector)

_CACHE = {}


# ---------------------------------------------------------------- device IR
def _build_nc():
    from contextlib import ExitStack

    import concourse.bacc as bacc
    import concourse.mybir as mybir
    import concourse.tile as tile

    dt = mybir.dt
    AF = mybir.ActivationFunctionType

    nc = bacc.Bacc("TRN2", target_bir_lowering=False, debug=False)

    xt_d = nc.dram_tensor("xt", [HT, P, S], dt.bfloat16, kind="ExternalInput")
    xq_d = nc.dram_tensor("xq", [HT, P, S_LOC], dt.bfloat16, kind="ExternalInput")
    wq_d = nc.dram_tensor("wq", [NH, P, HT, HD], dt.bfloat16, kind="ExternalInput")
    wk_d = nc.dram_tensor("wk", [NKV, P, HT, HD], dt.bfloat16, kind="ExternalInput")
    wv_d = nc.dram_tensor("wv", [HT, P, KVC], dt.bfloat16, kind="ExternalInput")
    wp_d = nc.dram_tensor("wp", [HT, P, H], dt.bfloat16, kind="ExternalInput")
    qcos_d = nc.dram_tensor("qcos", [P, S_LOC], dt.bfloat16, kind="ExternalInput")
    qsin_d = nc.dram_tensor("qsin", [P, S_LOC], dt.bfloat16, kind="ExternalInput")
    kcos_d = nc.dram_tensor("kcos", [P, S], dt.bfloat16, kind="ExternalInput")
    ksin_d = nc.dram_tensor("ksin", [P, S], dt.bfloat16, kind="ExternalInput")
    edge_d = nc.dram_tensor("edge", [P, 2 * 64], dt.bfloat16, kind="ExternalInput")
    nw_d = nc.dram_tensor("nw", [P, H], dt.float32, kind="ExternalInput")
    out_d = nc.dram_tensor("out", [S_LOC, H], dt.float32, kind="ExternalOutput")

    with tile.TileContext(nc) as tc, ExitStack() as body:
        const = body.enter_context(tc.tile_pool(name="const", bufs=1))
        onesm = const.tile([P, P], dt.bfloat16)
        nc.vector.memset(onesm[:], 1.0)
        epsb = const.tile([P, 1], dt.float32)
        nc.vector.memset(epsb[:], RMS_EPS)
        edge = const.tile([P, 2 * 64], dt.bfloat16)

        # pools that live through phases A+B
        s_ab = body.enter_context(ExitStack())
        abp = s_ab.enter_context(tc.tile_pool(name="abp", bufs=1))
        xq = abp.tile([P, HT * S_LOC], dt.bfloat16)
        kT = abp.tile([P, NKV * S], dt.bfloat16)
        vv = abp.tile([P, (S // P) * KVC], dt.bfloat16)
        qcos = abp.tile([P, S_LOC], dt.bfloat16)
        qsin = abp.tile([P, S_LOC], dt.bfloat16)

        qtr = s_ab.enter_context(tc.tile_pool(name="qtr", bufs=3))
        wqr = s_ab.enter_context(tc.tile_pool(name="wqr", bufs=2))
        rpq = s_ab.enter_context(tc.tile_pool(name="rpq", bufs=2))

        # deferred per-(head,t) finalize: denominator matmuls + reciprocal +
        # yT normalization, emitted ~one tile later so the in-order PE queue
        # never waits on the cross-engine reduction chain.
        pending = [None]

        def flush_pending():
            if pending[0] is not None:
                fin, pending[0] = pending[0], None
                fin()

        qstate = {}

        def q_part(hq, t, pspool, psname):
            # one i-slot group of head hq's q projection (16 MMs + evict)
            if t == 0:
                wqt = wqr.tile([P, HT * HD], dt.bfloat16, name="wqt")
                nc.sync.dma_start(
                    wqt[:].rearrange("p (t m) -> p t m", t=HT), wq_d.ap()[hq]
                )
                stg = rpq.tile([P, S_LOC], dt.bfloat16, name="stgq",
                               tag="stgq", bufs=1)
                qstate[hq] = (wqt, stg)
            wqt, stg = qstate[hq]
            ps = pspool.tile([P, IT], dt.float32, name=psname)
            for h in range(HT):
                nc.tensor.matmul(
                    ps[:],
                    wqt[:, h * HD : (h + 1) * HD],
                    xq[:, h * S_LOC + t * IT : h * S_LOC + (t + 1) * IT],
                    start=(h == 0),
                    stop=(h == HT - 1),
                )
            # eviction on DVE: keeps it out of the exp-laden ACT FIFO
            nc.vector.tensor_copy(stg[:, t * IT : (t + 1) * IT], ps[:])

        def rope_finish(hq):
            _, stg = qstate.pop(hq)
            qt = qtr.tile([P, S_LOC], dt.bfloat16, name="qt")
            t1 = rpq.tile([HH, S_LOC], dt.bfloat16, name="rt1q", tag="rt1q", bufs=1)
            t2 = rpq.tile([HH, S_LOC], dt.bfloat16, name="rt2q", tag="rt2q", bufs=1)
            nc.vector.tensor_mul(t1[:], stg[0:HH, :], qcos[0:HH, :])
            nc.vector.tensor_mul(t2[:], stg[HH:P, :], qsin[HH:P, :])
            nc.vector.tensor_sub(qt[0:HH, :], t1[:], t2[:])
            nc.vector.tensor_mul(t1[:], stg[HH:P, :], qcos[HH:P, :])
            nc.vector.tensor_mul(t2[:], stg[0:HH, :], qsin[0:HH, :])
            nc.vector.tensor_add(qt[HH:P, :], t1[:], t2[:])
            return qt

        # ---------------- phase A: k/v projections ---------------------
        with ExitStack() as phA:
            xtp = phA.enter_context(tc.tile_pool(name="xtp", bufs=1))
            xt = xtp.tile([P, HT * S], dt.bfloat16)
            wkp = phA.enter_context(tc.tile_pool(name="wkp", bufs=1))
            wk = wkp.tile([P, NKV * HT * HD], dt.bfloat16)
            kcs = phA.enter_context(tc.tile_pool(name="kcs", bufs=1))
            kcos = kcs.tile([P, S], dt.bfloat16)
            ksin = kcs.tile([P, S], dt.bfloat16)
            # weights + first x tiles first: they gate the first matmul
            def wk_chunk(fk, hc, n=8):
                nc.sync.dma_start(
                    wk[
                        :, fk * HT * HD + hc * HD : fk * HT * HD + (hc + n) * HD
                    ].rearrange("p (t m) -> p t m", t=n),
                    wk_d.ap()[fk, :, hc : hc + n],
                )

            wk_chunk(0, 0)
            nc.sync.dma_start(xt[:, 0:S], xt_d.ap()[0])
            wk_chunk(1, 0)
            nc.sync.dma_start(xt[:, S : 2 * S], xt_d.ap()[1])
            wk_chunk(0, 8)
            wk_chunk(1, 8)
            for h in range(2, HT):
                nc.sync.dma_start(xt[:, h * S : (h + 1) * S], xt_d.ap()[h])
            nc.sync.dma_start(kcos[:], kcos_d.ap())
            nc.sync.dma_start(ksin[:], ksin_d.ap())
            for fk in (2, 3):
                nc.sync.dma_start(
                    wk[:, fk * HT * HD : (fk + 1) * HT * HD].rearrange(
                        "p (t m) -> p t m", t=HT
                    ),
                    wk_d.ap()[fk],
                )
            wvp = phA.enter_context(tc.tile_pool(name="wvp", bufs=1))
            wv = wvp.tile([P, HT * KVC], dt.bfloat16)
            nc.sync.dma_start(
                wv[:].rearrange("p (t f) -> p t f", t=HT),
                wv_d.ap().rearrange("t p f -> p t f"),
            )
            for h in range(HT):
                nc.sync.dma_start(
                    xq[:, h * S_LOC : (h + 1) * S_LOC], xq_d.ap()[h]
                )
            nc.sync.dma_start(qcos[:], qcos_d.ap())
            nc.sync.dma_start(qsin[:], qsin_d.ap())
            nc.sync.dma_start(edge[:], edge_d.ap())

            psA = phA.enter_context(tc.tile_pool(name="psA", bufs=8, space="PSUM"))
            rpk = phA.enter_context(tc.tile_pool(name="rpk", bufs=2))

            def rope_evict_k(ps, fk, sc):
                c0 = fk * S + sc * IT
                cs = kcos[:, sc * IT : (sc + 1) * IT]
                sn = ksin[:, sc * IT : (sc + 1) * IT]
                stg = rpk.tile([P, IT], dt.bfloat16, name="stgk")
                nc.scalar.activation(stg[:], ps[:], AF.Copy)
                t1 = rpk.tile([HH, IT], dt.bfloat16, name="rt1k", tag="rt1k", bufs=1)
                t2 = rpk.tile([HH, IT], dt.bfloat16, name="rt2k", tag="rt2k", bufs=1)
                nc.vector.tensor_mul(t1[:], stg[0:HH, :], cs[0:HH, :])
                nc.vector.tensor_mul(t2[:], stg[HH:P, :], sn[HH:P, :])
                nc.vector.tensor_sub(kT[0:HH, c0 : c0 + IT], t1[:], t2[:])
                nc.vector.tensor_mul(t1[:], stg[HH:P, :], cs[HH:P, :])
                nc.vector.tensor_mul(t2[:], stg[0:HH, :], sn[0:HH, :])
                nc.vector.tensor_add(kT[HH:P, c0 : c0 + IT], t1[:], t2[:])

            # wave A: 8 interleaved k groups, h-outer (DMA-paced)
            groups8 = [(fk, sc) for fk in (0, 1) for sc in range(S // IT)]
            kps = [psA.tile([P, IT], dt.float32, name="aps") for _ in groups8]
            for h in range(HT):
                for g, (fk, sc) in enumerate(groups8):
                    nc.tensor.matmul(
                        kps[g][:],
                        wk[:, fk * HT * HD + h * HD : fk * HT * HD + (h + 1) * HD],
                        xt[:, h * S + sc * IT : h * S + (sc + 1) * IT],
                        start=(h == 0),
                        stop=(h == HT - 1),
                    )
            for g, (fk, sc) in enumerate(groups8):
                rope_evict_k(kps[g], fk, sc)

            # wave B: remaining k groups, group-outer
            for fk in (2, 3):
                for sc in range(S // IT):
                    ps = psA.tile([P, IT], dt.float32, name="aps")
                    for h in range(HT):
                        nc.tensor.matmul(
                            ps[:],
                            wk[:, fk * HT * HD + h * HD : fk * HT * HD + (h + 1) * HD],
                            xt[:, h * S + sc * IT : h * S + (sc + 1) * IT],
                            start=(h == 0),
                            stop=(h == HT - 1),
                        )
                    rope_evict_k(ps, fk, sc)

            # wave C: v projection (natural [s, f] layout)
            for sv in range(S // P):
                ps = psA.tile([P, KVC], dt.float32, name="aps")
                for h in range(HT):
                    nc.tensor.matmul(
                        ps[:],
                        xt[:, h * S + sv * P : h * S + (sv + 1) * P],
                        wv[:, h * KVC : (h + 1) * KVC],
                        start=(h == 0),
                        stop=(h == HT - 1),
                    )
                nc.scalar.activation(
                    vv[:, sv * KVC : (sv + 1) * KVC], ps[:], AF.Copy
                )

            # wave D: q-proj heads 0,1 to prime the phase-B pipeline
            qtiles = {}
            for hq in (0, 1):
                q_part(hq, 0, psA, "aps")
                q_part(hq, 1, psA, "aps")
                qtiles[hq] = rope_finish(hq)

        # ---------------- phase B: q-proj + attention pipeline ----------
        # yT lives B..C
        latep = body.enter_context(tc.tile_pool(name="latep", bufs=1, side="right"))
        yT = latep.tile([P, NH * S_LOC], dt.bfloat16)

        with ExitStack() as phB:
            wpap = body.enter_context(tc.tile_pool(name="wpap", bufs=1, side="right"))
            wpa = wpap.tile([P, WPA * H], dt.bfloat16)
            for i in range(WPA):
                nc.sync.dma_start(wpa[:, i * H : (i + 1) * H], wp_d.ap()[i])

            pB = phB.enter_context(tc.tile_pool(name="pB", bufs=1, space="PSUM"))
            prp = phB.enter_context(tc.tile_pool(name="prp", bufs=4))
            dsp = phB.enter_context(tc.tile_pool(name="dsp", bufs=3))
            qdp = phB.enter_context(tc.tile_pool(name="qdp", bufs=5))
            recp = phB.enter_context(tc.tile_pool(name="recp", bufs=2))
            sps = phB.enter_context(tc.tile_pool(name="sps", bufs=2, space="PSUM"))

            ge = nc.gpsimd if GP_EDGES else nc.vector
            gq = nc.gpsimd if GP_QUADS else nc.vector
            edge2 = edge[:].rearrange("p (u v) -> p u v", u=2)
            EW = IT - 64  # stride between the two edge blocks of a pair

            def attention(hq):
                kvh = hq // G
                qt = qtiles.pop(hq)
                for t in range(NT_I):
                    qsl = qt[:, t * IT : (t + 1) * IT]
                    yps = pB.tile([P, IT], dt.float32, name="yps", tag="yps", bufs=2)
                    dss = []
                    qds = []
                    state = {"y_first": True, "pend": None,
                             "flush_at": 1, "defers": 0}

                    def flush_quad(ragged):
                        if len(dss) >= 2:
                            (da, lo_a), (db, lo_b) = dss[0], dss[1]
                            qd = qdp.tile([P, IT], dt.bfloat16, name="qds")
                            if ragged and lo_b:
                                nc.vector.tensor_copy(
                                    qd[:, lo_a:lo_b], da[:, lo_a:lo_b]
                                )
                            gq.tensor_add(
                                qd[:, lo_b:IT], da[:, lo_b:IT], db[:, lo_b:IT]
                            )
                            qds.append((qd, lo_a))
                            del dss[:]

                    def consume_pair(info, last):
                        # y-MMs + denominator partial for a pair, emitted one
                        # pair late so their semaphores clear before the PE
                        # reaches them (keeps the MM stream at full rate)
                        pr, j_hi, j_lo, ohi, olo = info
                        ds = dsp.tile([P, IT], dt.bfloat16, name="ds")
                        if ohi:  # band pair: ragged union [olo, IT)
                            nc.vector.tensor_copy(
                                ds[:, olo:ohi], pr[:, IT + olo : IT + ohi]
                            )
                            nc.vector.tensor_add(
                                ds[:, ohi:IT], pr[:, ohi:IT], pr[:, IT + ohi : 2 * IT]
                            )
                        else:
                            # full pairs sit early in the tile: their extra
                            # gpsimd latency is hidden, and it unloads DVE
                            nc.gpsimd.tensor_add(
                                ds[:, 0:IT], pr[:, 0:IT], pr[:, IT : 2 * IT]
                            )
                        dss.append((ds, olo))
                        flush_quad(ragged=bool(ohi))
                        nc.tensor.matmul(
                            yps[:, ohi:IT],
                            vv[:, j_hi * KVC + kvh * HD : j_hi * KVC + (kvh + 1) * HD],
                            pr[:, ohi:IT],
                            start=state["y_first"],
                            stop=False,
                        )
                        state["y_first"] = False
                        nc.tensor.matmul(
                            yps[:, olo:IT],
                            vv[:, j_lo * KVC + kvh * HD : j_lo * KVC + (kvh + 1) * HD],
                            pr[:, IT + olo : 2 * IT],
                            start=False,
                            stop=last,
                        )

                    def defer(info):
                        if state["pend"] is not None:
                            consume_pair(state["pend"], last=False)
                        state["pend"] = info
                        state["defers"] += 1
                        if state["defers"] == state["flush_at"]:
                            # previous (head,t)'s denominator + normalize,
                            # a pair or two into this tile's stream
                            flush_pending()

                    # full past tiles, in pairs
                    for jp in range(t * JB // 2):
                        j0 = 2 * jp
                        sp = sps.tile([P, 2 * IT], dt.float32, name="sps")
                        for u in (0, 1):
                            nc.tensor.matmul(
                                sp[:, u * IT : (u + 1) * IT],
                                kT[:, kvh * S + (j0 + u) * P : kvh * S + (j0 + u + 1) * P],
                                qsl,
                                start=True,
                                stop=True,
                            )
                        pr = prp.tile([P, 2 * IT], dt.bfloat16, name="pr")
                        nc.scalar.activation(
                            pr[:, 0 : 2 * IT], sp[:], AF.Exp, scale=SCALE
                        )
                        defer((pr, j0, j0 + 1, 0, 0))

                    # diagonal band tiles: ragged pairs (descending order)
                    for bp in range(JB // 2):
                        jlo, jhi = 2 * bp, 2 * bp + 1
                        j_lo, j_hi = t * JB + jlo, t * JB + jhi
                        olo, ohi = 64 * jlo, 64 * jhi
                        sp = sps.tile([P, 2 * IT], dt.float32, name="sps")
                        nc.tensor.matmul(
                            sp[:, ohi:IT],
                            kT[:, kvh * S + j_hi * P : kvh * S + (j_hi + 1) * P],
                            qt[:, t * IT + ohi : (t + 1) * IT],
                            start=True,
                            stop=True,
                        )
                        nc.tensor.matmul(
                            sp[:, IT + olo : 2 * IT],
                            kT[:, kvh * S + j_lo * P : kvh * S + (j_lo + 1) * P],
                            qt[:, t * IT + olo : (t + 1) * IT],
                            start=True,
                            stop=True,
                        )
                        pr = prp.tile([P, 2 * IT], dt.bfloat16, name="pr")
                        nc.scalar.activation(
                            pr[:, ohi : 2 * IT], sp[:, ohi : 2 * IT],
                            AF.Exp, scale=SCALE,
                        )
                        # both 64-wide edge blocks ([ohi,+64) and [IT+olo,+64),
                        # stride EW apart) in one strided op; X shifts the
                        # window so the 2*EW base slice stays inside the tile
                        X = max(0, ohi - (2 * IT - 2 * EW))
                        eap = pr[:, ohi - X : ohi - X + 2 * EW].rearrange(
                            "p (u v) -> p u v", u=2
                        )[:, :, X : X + 64]
                        ge.tensor_mul(eap, eap, edge2)
                        defer((pr, j_hi, j_lo, ohi, olo))
                    consume_pair(state["pend"], last=True)

                    def fin(hq=hq, t=t, yps=yps, qds=tuple(qds)):
                        dps = pB.tile([P, IT], dt.float32, name="dps",
                                      tag="dps", bufs=1)
                        for i, (qd, lo) in enumerate(qds):
                            nc.tensor.matmul(
                                dps[:, lo:IT], onesm[:], qd[:, lo:IT],
                                start=(i == 0), stop=(i == len(qds) - 1),
                            )
                        rec = recp.tile([P, IT], dt.float32, name="rec")
                        nc.vector.reciprocal_approx_fast(rec[:], dps[:])
                        nc.vector.tensor_mul(
                            yT[:, hq * S_LOC + t * IT : hq * S_LOC + (t + 1) * IT],
                            yps[:],
                            rec[:],
                        )

                    pending[0] = fin

            # software pipeline: head hq's attention is flanked by the
            # (hq+2) q-proj t1-group + rope and the (hq+3) t0-group, giving
            # each deferred finalize ~7us of independent PE work to hide
            # the trailing exp->edge->reduce chain of the previous head.
            q_part(2, 0, pB, "qps")
            for hq in range(NH):
                attention(hq)
                if hq + 2 < NH:
                    q_part(hq + 2, 1, pB, "qps")
                    qtiles[hq + 2] = rope_finish(hq + 2)
                if hq + 3 < NH:
                    q_part(hq + 3, 0, pB, "qps")
            flush_pending()

        s_ab.close()  # free xt / kT / vv / q pools before the projection phase

        # ---------------- phase C: out projection + rmsnorm -------------
        with ExitStack() as phC:
            wpbp = phC.enter_context(tc.tile_pool(name="wpbp", bufs=1))
            wpc = wpbp.tile([P, (HT - WPA) * H], dt.bfloat16)
            for i in range(HT - WPA):
                nc.sync.dma_start(
                    wpc[:, i * H : (i + 1) * H], wp_d.ap()[WPA + i]
                )
            nwp = phC.enter_context(tc.tile_pool(name="nwp", bufs=1))
            nw = nwp.tile([P, H], dt.float32)
            nc.sync.dma_start(nw[:], nw_d.ap())

            outp = phC.enter_context(tc.tile_pool(name="outp", bufs=2))
            sqp = phC.enter_context(tc.tile_pool(name="sqp", bufs=2))
            smp = phC.enter_context(tc.tile_pool(name="smp", bufs=2))
            po = phC.enter_context(tc.tile_pool(name="po", bufs=8, space="PSUM"))

            mult = mybir.AluOpType.mult

            nslice = S_LOC // P
            for sl in range(nslice):
                # last slice runs o-outer so the norm chain pipelines with
                # the matmuls and the tail after the final matmul is short
                o_outer = sl == nslice - 1
                pso = [po.tile([P, OT], dt.float32, name="pso") for _ in range(NO)]
                ot = outp.tile([P, H], dt.float32, name="ot")
                ssqs = []

                def chunk_post(o):
                    sq = sqp.tile([P, OT], dt.float32, name="sq")
                    sso = smp.tile([P, 1], dt.float32, name="sso", tag="sso", bufs=8)
                    nc.scalar.activation(
                        sq[:], pso[o][:], AF.Square, accum_out=sso[:]
                    )
                    nc.scalar.activation(
                        ot[:, o * OT : (o + 1) * OT], pso[o][:], AF.Copy
                    )
                    ssqs.append(sso)

                lhss = [
                    yT[:, h * S_LOC + sl * P : h * S_LOC + (sl + 1) * P]
                    for h in range(HT)
                ]
                def wslice(h, o):
                    if h < WPA:
                        return wpa[:, h * H + o * OT : h * H + (o + 1) * OT]
                    hh = h - WPA
                    return wpc[:, hh * H + o * OT : hh * H + (o + 1) * OT]

                if o_outer:
                    for o in range(NO):
                        for h in range(HT):
                            nc.tensor.matmul(
                                pso[o][:],
                                lhss[h],
                                wslice(h, o),
                                start=(h == 0),
                                stop=(h == HT - 1),
                            )
                        chunk_post(o)
                else:
                    for h in range(HT):
                        for o in range(NO):
                            nc.tensor.matmul(
                                pso[o][:],
                                lhss[h],
                                wslice(h, o),
                                start=(h == 0),
                                stop=(h == HT - 1),
                            )
                    for o in range(NO):
                        chunk_post(o)
                sa = smp.tile([P, 1], dt.float32, name="sa")
                sb = smp.tile([P, 1], dt.float32, name="sb")
                nc.vector.tensor_add(sa[:], ssqs[0][:], ssqs[1][:])
                nc.vector.tensor_add(sb[:], ssqs[2][:], ssqs[3][:])
                ssq = smp.tile([P, 1], dt.float32, name="ssq")
                nc.vector.tensor_add(ssq[:], sa[:], sb[:])
                rms = smp.tile([P, 1], dt.float32, name="rms")
                nc.scalar.activation(
                    rms[:], ssq[:], AF.Sqrt, bias=epsb[:], scale=1.0 / H
                )
                rr = smp.tile([P, 1], dt.float32, name="rr")
                nc.vector.reciprocal(rr[:], rms[:])
                for half in range(2):
                    for o in (2 * half, 2 * half + 1):
                        nc.vector.scalar_tensor_tensor(
                            ot[:, o * OT : (o + 1) * OT],
                            ot[:, o * OT : (o + 1) * OT],
                            rr[:],
                            nw[:, o * OT : (o + 1) * OT],
                            mult,
                            mult,
                        )
                    nc.sync.dma_start(
                        out_d.ap()[sl * P : (sl + 1) * P, half * H // 2 : (half + 1) * H // 2],
                        ot[:, half * H // 2 : (half + 1) * H // 2],
                    )

    nc.compile()
    return nc


# ---------------------------------------------------------------- host side
def _host_shared(w_attn, w_proj, norm_w):
    """Core-independent packed tensors."""
    f32 = np.float32

    def perm_halves(w):  # [H, n, HD] even/odd pairs -> halves
        return np.concatenate([w[..., 0::2], w[..., 1::2]], axis=-1)

    wq = perm_halves(w_attn[:, :H].reshape(H, NH, HD))
    wq = np.ascontiguousarray(
        wq.reshape(HT, P, NH, HD).transpose(2, 1, 0, 3)
    ).astype(BF16)
    wk = perm_halves(w_attn[:, H : H + KVC].reshape(H, NKV, HD))
    wk = np.ascontiguousarray(
        wk.reshape(HT, P, NKV, HD).transpose(2, 1, 0, 3)
    ).astype(BF16)
    wv = np.ascontiguousarray(
        w_attn[:, H + KVC :].reshape(HT, P, KVC)
    ).astype(BF16)
    wp = np.ascontiguousarray(w_proj.reshape(HT, P, H)).astype(BF16)

    p, f = np.meshgrid(np.arange(P), np.arange(64), indexing="ij")
    # parity 0: query col f (global row 2f within the 64-block diagonal)
    edge0 = (2 * f >= p).astype(BF16)
    # parity 1: query 2f+1 vs key (p^1) (pair-swapped x columns)
    edge1 = (2 * f + 1 >= (p ^ 1)).astype(BF16)
    # duplicated side by side: one strided op masks both blocks of a pair
    edge0 = np.ascontiguousarray(np.concatenate([edge0, edge0], axis=1))
    edge1 = np.ascontiguousarray(np.concatenate([edge1, edge1], axis=1))

    nw = np.ascontiguousarray(
        np.broadcast_to(norm_w.astype(f32), (P, H))
    )
    return wq, wk, wv, wp, (edge0, edge1), nw


def _cos_sin(pos):
    f32 = np.float32
    inv = 1.0 / (
        10000.0 ** (np.arange(0, HD, 2, dtype=f32) / f32(HD))
    )
    ang = inv[:, None].astype(f32) * pos[None, :].astype(f32)  # [HH, N]
    c, s = np.cos(ang).astype(BF16), np.sin(ang).astype(BF16)
    # duplicated across both partition halves (walrus wants equal base
    # partitions for SBUF tensor-tensor inputs)
    return (
        np.ascontiguousarray(np.concatenate([c, c], axis=0)),
        np.ascontiguousarray(np.concatenate([s, s], axis=0)),
    )


def make_in_maps(x, w_attn, w_proj, norm_w):
    x = np.asarray(x, dtype=np.float32)
    w_attn = np.asarray(w_attn, dtype=np.float32)
    w_proj = np.asarray(w_proj, dtype=np.float32)
    norm_w = np.asarray(norm_w, dtype=np.float32)

    wq, wk, wv, wp, (edge0, edge1), nw = _host_shared(w_attn, w_proj, norm_w)

    kc0, ks0 = _cos_sin(np.arange(S, dtype=np.float32))          # parity 0
    # parity 1: column j holds global row j^1 (pair-swapped x columns)
    kc1, ks1 = _cos_sin((np.arange(S) ^ 1).astype(np.float32))
    qc0, qs0 = _cos_sin(2.0 * np.arange(S_LOC, dtype=np.float32))
    qc1, qs1 = _cos_sin(2.0 * np.arange(S_LOC, dtype=np.float32) + 1.0)

    in_maps = []
    for c in range(N_CORES):
        b, par = c // 2, c % 2
        xt = x[b].T.astype(BF16)
        if par:
            xt = xt[:, np.arange(S) ^ 1]  # swap adjacent column pairs
        # parity-packed contiguous copy for the q projection
        xq = np.ascontiguousarray(
            xt[:, 0::2].reshape(HT, P, S_LOC)
        )
        xt = np.ascontiguousarray(xt.reshape(HT, P, S))
        in_maps.append(
            {
                "xt": xt,
                "xq": xq,
                "wq": wq,
                "wk": wk,
                "wv": wv,
                "wp": wp,
                "qcos": qc1 if par else qc0,
                "qsin": qs1 if par else qs0,
                "kcos": kc1 if par else kc0,
                "ksin": ks1 if par else ks0,
                "edge": edge1 if par else edge0,
                "nw": nw,
            }
        )
    return in_maps


def assemble_out(results):
    out = np.empty((B, S, H), dtype=np.float32)
    for c in range(N_CORES):
        b, par = c // 2, c % 2
        out[b, par::2, :] = results[c]["out"]
    return out


def kernel(x, w_attn, w_proj, norm_w):
    from concourse import bass_utils

    if "nc" not in _CACHE:
        _CACHE["nc"] = _build_nc()
    nc = _CACHE["nc"]

    in_maps = make_in_maps(x, w_attn, w_proj, norm_w)
    res = bass_utils.run_bass_kernel_spmd(
        nc, in_maps, core_ids=list(range(N_CORES))
    )
    return assemble_out(res.results)


# revision 56
# speedup vs baseline: 1.0143x; 1.0143x over previous
"""Trainium2 Bass kernel for nn_Attention_8366596292664.

Dense transformer block: qkv proj -> RoPE -> GQA causal attention ->
out proj -> RMSNorm.  B=4, S=2048, H=2048, 16 heads (hd=128), 4 KV heads.

Sharding: 8 cores = (4 batches) x (2 interleaved query-row parities).
Core (b, par) computes the full block for query rows {par, par+2, ...} of
batch b.  Interleaving the query rows by parity makes the causal structure
identical on every core, so one SPMD program serves all 8 cores; the
parity enters only through the data (a 1-column roll of x^T, cos/sin
tables, and the output row scatter).

Structure (v2):
  Phase A: k/v projections.  x^T streams in as 16 per-h-tile DMAs while
    8 interleaved PSUM accumulation groups consume the tiles h-outer, so
    the PE starts ~3us into the kernel instead of waiting for the full
    8.4MB transfer.  Ends with q-proj of heads 0-1 to prime phase B.
  Phase B: per-head software pipeline [q-proj head h+2 | attention head h].
    Overlaps the PE-heavy projection with the ACT-heavy softmax.  The
    causal diagonal band is triangularized: ragged matmuls with 64-column
    offsets skip ~30% of score/AV/exp work; a single 64-col edge mask +
    memsets replace the old full mask tensor.  Edge masking and the
    denominator quad-reduction run on the otherwise idle GpSimd engine.
  Phase C: out-proj + RMSNorm.  w_proj is half-prefetched during phase B
    (no PE gap at the boundary); final scaling is one fused
    scalar_tensor_tensor (ot*rr)*norm_w per chunk.
"""

import numpy as np
import ml_dtypes

BF16 = ml_dtypes.bfloat16

# ---------------------------------------------------------------- config
P = 128          # partitions
HD = 128         # head dim
HH = HD // 2     # rope half
G = 4            # GQA group size

B = 4
S = 2048
H = 2048
N_CORES = 8

NH = H // HD          # 16 q heads
NKV = NH // G         # 4 kv heads
KVC = NKV * HD        # 512 kv columns
HT = H // P           # 16 h-tiles (contraction tiles)
S_LOC = S // 2        # 1024 local q rows per core
IT = 512              # i-tile (queries per score tile, = 1 psum bank fp32)
NT_I = S_LOC // IT    # 2 i-slots
SPAN = S // NT_I      # 1024 global rows per slot
JB = SPAN // P        # 8 j-tiles in the diagonal band of each slot
OT = 512              # output-proj column tile
NO = H // OT          # 4
WPA = 8               # wp h-tiles prefetched during phase B

RMS_EPS = 1e-6
SCALE = 1.0 / float(np.sqrt(np.float32(HD)))

# engine routing knobs
GP_EDGES = True       # edge-mask muls on gpsimd (else vector)
GP_QUADS = False      # denominator quad adds on gpsimd (else vector)

_CACHE = {}


# ---------------------------------------------------------------- device IR
def _build_nc():
    from contextlib import ExitStack

    import concourse.bacc as bacc
    import concourse.mybir as mybir
    import concourse.tile as tile

    dt = mybir.dt
    AF = mybir.ActivationFunctionType

    nc = bacc.Bacc("TRN2", target_bir_lowering=False, debug=False)

    xt_d = nc.dram_tensor("xt", [HT, P, S], dt.bfloat16, kind="ExternalInput")
    xq_d = nc.dram_tensor("xq", [HT, P, S_LOC], dt.bfloat16, kind="ExternalInput")
    wq_d = nc.dram_tensor("wq", [NH, P, HT, HD], dt.bfloat16, kind="ExternalInput")
    wk_d = nc.dram_tensor("wk", [NKV, P, HT, HD], dt.bfloat16, kind="ExternalInput")
    wv_d = nc.dram_tensor("wv", [HT, P, KVC], dt.bfloat16, kind="ExternalInput")
    wp_d = nc.dram_tensor("wp", [HT, P, H], dt.bfloat16, kind="ExternalInput")
    qcos_d = nc.dram_tensor("qcos", [P, S_LOC], dt.bfloat16, kind="ExternalInput")
    qsin_d = nc.dram_tensor("qsin", [P, S_LOC], dt.bfloat16, kind="ExternalInput")
    kcos_d = nc.dram_tensor("kcos", [P, S], dt.bfloat16, kind="ExternalInput")
    ksin_d = nc.dram_tensor("ksin", [P, S], dt.bfloat16, kind="ExternalInput")
    edge_d = nc.dram_tensor("edge", [P, 2 * 64], dt.bfloat16, kind="ExternalInput")
    nw_d = nc.dram_tensor("nw", [P, H], dt.float32, kind="ExternalInput")
    out_d = nc.dram_tensor("out", [S_LOC, H], dt.float32, kind="ExternalOutput")

    with tile.TileContext(nc) as tc, ExitStack() as body:
        const = body.enter_context(tc.tile_pool(name="const", bufs=1))
        onesm = const.tile([P, P], dt.bfloat16)
        nc.vector.memset(onesm[:], 1.0)
        epsb = const.tile([P, 1], dt.float32)
        nc.vector.memset(epsb[:], RMS_EPS)
        edge = const.tile([P, 2 * 64], dt.bfloat16)

        # pools that live through phases A+B
        s_ab = body.enter_context(ExitStack())
        abp = s_ab.enter_context(tc.tile_pool(name="abp", bufs=1))
        xq = abp.tile([P, HT * S_LOC], dt.bfloat16)
        kT = abp.tile([P, NKV * S], dt.bfloat16)
        vv = abp.tile([P, (S // P) * KVC], dt.bfloat16)
        qcos = abp.tile([P, S_LOC], dt.bfloat16)
        qsin = abp.tile([P, S_LOC], dt.bfloat16)

        qtr = s_ab.enter_context(tc.tile_pool(name="qtr", bufs=3))
        wqr = s_ab.enter_context(tc.tile_pool(name="wqr", bufs=2))
        rpq = s_ab.enter_context(tc.tile_pool(name="rpq", bufs=2))

        # deferred per-(head,t) finalize: denominator matmuls + reciprocal +
        # yT normalization, emitted ~one tile later so the in-order PE queue
        # never waits on the cross-engine reduction chain.
        pending = [None]

        def flush_pending():
            if pending[0] is not None:
                fin, pending[0] = pending[0], None
                fin()

        qstate = {}

        def q_part(hq, t, pspool, psname):
            # one i-slot group of head hq's q projection (16 MMs + evict)
            if t == 0:
                wqt = wqr.tile([P, HT * HD], dt.bfloat16, name="wqt")
                nc.sync.dma_start(
                    wqt[:].rearrange("p (t m) -> p t m", t=HT), wq_d.ap()[hq]
                )
                stg = rpq.tile([P, S_LOC], dt.bfloat16, name="stgq",
                               tag="stgq", bufs=1)
                qstate[hq] = (wqt, stg)
            wqt, stg = qstate[hq]
            ps = pspool.tile([P, IT], dt.float32, name=psname)
            for h in range(HT):
                nc.tensor.matmul(
                    ps[:],
                    wqt[:, h * HD : (h + 1) * HD],
                    xq[:, h * S_LOC + t * IT : h * S_LOC + (t + 1) * IT],
                    start=(h == 0),
                    stop=(h == HT - 1),
                )
            # eviction on DVE: keeps it out of the exp-laden ACT FIFO
            nc.vector.tensor_copy(stg[:, t * IT : (t + 1) * IT], ps[:])

        def rope_finish(hq):
            _, stg = qstate.pop(hq)
            qt = qtr.tile([P, S_LOC], dt.bfloat16, name="qt")
            t1 = rpq.tile([HH, S_LOC], dt.bfloat16, name="rt1q", tag="rt1q", bufs=1)
            t2 = rpq.tile([HH, S_LOC], dt.bfloat16, name="rt2q", tag="rt2q", bufs=1)
            nc.vector.tensor_mul(t1[:], stg[0:HH, :], qcos[0:HH, :])
            nc.vector.tensor_mul(t2[:], stg[HH:P, :], qsin[HH:P, :])
            nc.vector.tensor_sub(qt[0:HH, :], t1[:], t2[:])
            nc.vector.tensor_mul(t1[:], stg[HH:P, :], qcos[HH:P, :])
            nc.vector.tensor_mul(t2[:], stg[0:HH, :], qsin[0:HH, :])
            nc.vector.tensor_add(qt[HH:P, :], t1[:], t2[:])
            return qt

        # ---------------- phase A: k/v projections ---------------------
        with ExitStack() as phA:
            xtp = phA.enter_context(tc.tile_pool(name="xtp", bufs=1))
            xt = xtp.tile([P, HT * S], dt.bfloat16)
            wkp = phA.enter_context(tc.tile_pool(name="wkp", bufs=1))
            wk = wkp.tile([P, NKV * HT * HD], dt.bfloat16)
            kcs = phA.enter_context(tc.tile_pool(name="kcs", bufs=1))
            kcos = kcs.tile([P, S], dt.bfloat16)
            ksin = kcs.tile([P, S], dt.bfloat16)
            # weights + first x tiles first: they gate the first matmul
            def wk_chunk(fk, hc, n=8):
                nc.sync.dma_start(
                    wk[
                        :, fk * HT * HD + hc * HD : fk * HT * HD + (hc + n) * HD
                    ].rearrange("p (t m) -> p t m", t=n),
                    wk_d.ap()[fk, :, hc : hc + n],
                )

            wk_chunk(0, 0)
            nc.sync.dma_start(xt[:, 0:S], xt_d.ap()[0])
            wk_chunk(1, 0)
            nc.sync.dma_start(xt[:, S : 2 * S], xt_d.ap()[1])
            wk_chunk(0, 8)
            wk_chunk(1, 8)
            for h in range(2, HT):
                nc.sync.dma_start(xt[:, h * S : (h + 1) * S], xt_d.ap()[h])
            nc.sync.dma_start(kcos[:], kcos_d.ap())
            nc.sync.dma_start(ksin[:], ksin_d.ap())
            for fk in (2, 3):
                nc.sync.dma_start(
                    wk[:, fk * HT * HD : (fk + 1) * HT * HD].rearrange(
                        "p (t m) -> p t m", t=HT
                    ),
                    wk_d.ap()[fk],
                )
            wvp = phA.enter_context(tc.tile_pool(name="wvp", bufs=1))
            wv = wvp.tile([P, HT * KVC], dt.bfloat16)
            nc.sync.dma_start(
                wv[:].rearrange("p (t f) -> p t f", t=HT),
                wv_d.ap().rearrange("t p f -> p t f"),
            )
            for h in range(HT):
                nc.sync.dma_start(
                    xq[:, h * S_LOC : (h + 1) * S_LOC], xq_d.ap()[h]
                )
            nc.sync.dma_start(qcos[:], qcos_d.ap())
            nc.sync.dma_start(qsin[:], qsin_d.ap())
            nc.sync.dma_start(edge[:], edge_d.ap())

            psA = phA.enter_context(tc.tile_pool(name="psA", bufs=8, space="PSUM"))
            rpk = phA.enter_context(tc.tile_pool(name="rpk", bufs=2))

            def rope_evict_k(ps, fk, sc):
                c0 = fk * S + sc * IT
                cs = kcos[:, sc * IT : (sc + 1) * IT]
                sn = ksin[:, sc * IT : (sc + 1) * IT]
                stg = rpk.tile([P, IT], dt.bfloat16, name="stgk")
                nc.scalar.activation(stg[:], ps[:], AF.Copy)
                t1 = rpk.tile([HH, IT], dt.bfloat16, name="rt1k", tag="rt1k", bufs=1)
                t2 = rpk.tile([HH, IT], dt.bfloat16, name="rt2k", tag="rt2k", bufs=1)
                nc.vector.tensor_mul(t1[:], stg[0:HH, :], cs[0:HH, :])
                nc.vector.tensor_mul(t2[:], stg[HH:P, :], sn[HH:P, :])
                nc.vector.tensor_sub(kT[0:HH, c0 : c0 + IT], t1[:], t2[:])
                nc.vector.tensor_mul(t1[:], stg[HH:P, :], cs[HH:P, :])
                nc.vector.tensor_mul(t2[:], stg[0:HH, :], sn[0:HH, :])
                nc.vector.tensor_add(kT[HH:P, c0 : c0 + IT], t1[:], t2[:])

            # wave A: 8 interleaved k groups, h-outer (DMA-paced)
            groups8 = [(fk, sc) for fk in (0, 1) for sc in range(S // IT)]
            kps = [psA.tile([P, IT], dt.float32, name="aps") for _ in groups8]
            for h in range(HT):
                for g, (fk, sc) in enumerate(groups8):
                    nc.tensor.matmul(
                        kps[g][:],
                        wk[:, fk * HT * HD + h * HD : fk * HT * HD + (h + 1) * HD],
                        xt[:, h * S + sc * IT : h * S + (sc + 1) * IT],
                        start=(h == 0),
                        stop=(h == HT - 1),
                    )
            for g, (fk, sc) in enumerate(groups8):
                rope_evict_k(kps[g], fk, sc)

            # wave B: remaining k groups, group-outer
            for fk in (2, 3):
                for sc in range(S // IT):
                    ps = psA.tile([P, IT], dt.float32, name="aps")
                    for h in range(HT):
                        nc.tensor.matmul(
                            ps[:],
                            wk[:, fk * HT * HD + h * HD : fk * HT * HD + (h + 1) * HD],
                            xt[:, h * S + sc * IT : h * S + (sc + 1) * IT],
                            start=(h == 0),
                            stop=(h == HT - 1),
                        )
                    rope_evict_k(ps, fk, sc)

            # wave C: v projection (natural [s, f] layout)
            for sv in range(S // P):
                ps = psA.tile([P, KVC], dt.float32, name="aps")
                for h in range(HT):
                    nc.tensor.matmul(
                        ps[:],
                        xt[:, h * S + sv * P : h * S + (sv + 1) * P],
                        wv[:, h * KVC : (h + 1) * KVC],
                        start=(h == 0),
                        stop=(h == HT - 1),
                    )
                nc.scalar.activation(
                    vv[:, sv * KVC : (sv + 1) * KVC], ps[:], AF.Copy
                )

            # wave D: q-proj heads 0,1 to prime the phase-B pipeline
            qtiles = {}
            for hq in (0, 1):
                q_part(hq, 0, psA, "aps")
                q_part(hq, 1, psA, "aps")
                qtiles[hq] = rope_finish(hq)

        # ---------------- phase B: q-proj + attention pipeline ----------
        # yT lives B..C
        latep = body.enter_context(tc.tile_pool(name="latep", bufs=1, side="right"))
        yT = latep.tile([P, NH * S_LOC], dt.bfloat16)

        with ExitStack() as phB:
            wpap = body.enter_context(tc.tile_pool(name="wpap", bufs=1, side="right"))
            wpa = wpap.tile([P, WPA * H], dt.bfloat16)
            for i in range(WPA):
                nc.sync.dma_start(wpa[:, i * H : (i + 1) * H], wp_d.ap()[i])

            pB = phB.enter_context(tc.tile_pool(name="pB", bufs=1, space="PSUM"))
            prp = phB.enter_context(tc.tile_pool(name="prp", bufs=4))
            dsp = phB.enter_context(tc.tile_pool(name="dsp", bufs=3))
            qdp = phB.enter_context(tc.tile_pool(name="qdp", bufs=5))
            recp = phB.enter_context(tc.tile_pool(name="recp", bufs=2))
            sps = phB.enter_context(tc.tile_pool(name="sps", bufs=2, space="PSUM"))

            ge = nc.gpsimd if GP_EDGES else nc.vector
            gq = nc.gpsimd if GP_QUADS else nc.vector
            edge2 = edge[:].rearrange("p (u v) -> p u v", u=2)
            EW = IT - 64  # stride between the two edge blocks of a pair

            def attention(hq):
                kvh = hq // G
                qt = qtiles.pop(hq)
                for t in range(NT_I):
                    qsl = qt[:, t * IT : (t + 1) * IT]
                    yps = pB.tile([P, IT], dt.float32, name="yps", tag="yps", bufs=2)
                    dss = []
                    qds = []
                    state = {"y_first": True, "pend": None,
                             "flush_at": 1, "defers": 0}

                    def flush_quad(ragged):
                        if len(dss) >= 2:
                            (da, lo_a), (db, lo_b) = dss[0], dss[1]
                            qd = qdp.tile([P, IT], dt.bfloat16, name="qds")
                            if ragged and lo_b:
                                nc.vector.tensor_copy(
                                    qd[:, lo_a:lo_b], da[:, lo_a:lo_b]
                                )
                            gq.tensor_add(
                                qd[:, lo_b:IT], da[:, lo_b:IT], db[:, lo_b:IT]
                            )
                            qds.append((qd, lo_a))
                            del dss[:]

                    def consume_pair(info, last):
                        # y-MMs + denominator partial for a pair, emitted one
                        # pair late so their semaphores clear before the PE
                        # reaches them (keeps the MM stream at full rate)
                        pr, j_hi, j_lo, ohi, olo = info
                        ds = dsp.tile([P, IT], dt.bfloat16, name="ds")
                        if ohi:  # band pair: ragged union [olo, IT)
                            nc.vector.tensor_copy(
                                ds[:, olo:ohi], pr[:, IT + olo : IT + ohi]
                            )
                            nc.vector.tensor_add(
                                ds[:, ohi:IT], pr[:, ohi:IT], pr[:, IT + ohi : 2 * IT]
                            )
                        else:
                            nc.vector.tensor_add(
                                ds[:, 0:IT], pr[:, 0:IT], pr[:, IT : 2 * IT]
                            )
                        dss.append((ds, olo))
                        flush_quad(ragged=bool(ohi))
                        nc.tensor.matmul(
                            yps[:, ohi:IT],
                            vv[:, j_hi * KVC + kvh * HD : j_hi * KVC + (kvh + 1) * HD],
                            pr[:, ohi:IT],
                            start=state["y_first"],
                            stop=False,
                        )
                        state["y_first"] = False
                        nc.tensor.matmul(
                            yps[:, olo:IT],
                            vv[:, j_lo * KVC + kvh * HD : j_lo * KVC + (kvh + 1) * HD],
                            pr[:, IT + olo : 2 * IT],
                            start=False,
                            stop=last,
                        )

                    def defer(info):
                        if state["pend"] is not None:
                            consume_pair(state["pend"], last=False)
                        state["pend"] = info
                        state["defers"] += 1
                        if state["defers"] == state["flush_at"]:
                            # previous (head,t)'s denominator + normalize,
                            # a pair or two into this tile's stream
                            flush_pending()

                    # full past tiles, in pairs
                    for jp in range(t * JB // 2):
                        j0 = 2 * jp
                        sp = sps.tile([P, 2 * IT], dt.float32, name="sps")
                        for u in (0, 1):
                            nc.tensor.matmul(
                                sp[:, u * IT : (u + 1) * IT],
                                kT[:, kvh * S + (j0 + u) * P : kvh * S + (j0 + u + 1) * P],
                                qsl,
                                start=True,
                                stop=True,
                            )
                        pr = prp.tile([P, 2 * IT], dt.bfloat16, name="pr")
                        nc.scalar.activation(
                            pr[:, 0 : 2 * IT], sp[:], AF.Exp, scale=SCALE
                        )
                        defer((pr, j0, j0 + 1, 0, 0))

                    # diagonal band tiles: ragged pairs (descending order)
                    for bp in range(JB // 2):
                        jlo, jhi = 2 * bp, 2 * bp + 1
                        j_lo, j_hi = t * JB + jlo, t * JB + jhi
                        olo, ohi = 64 * jlo, 64 * jhi
                        sp = sps.tile([P, 2 * IT], dt.float32, name="sps")
                        nc.tensor.matmul(
                            sp[:, ohi:IT],
                            kT[:, kvh * S + j_hi * P : kvh * S + (j_hi + 1) * P],
                            qt[:, t * IT + ohi : (t + 1) * IT],
                            start=True,
                            stop=True,
                        )
                        nc.tensor.matmul(
                            sp[:, IT + olo : 2 * IT],
                            kT[:, kvh * S + j_lo * P : kvh * S + (j_lo + 1) * P],
                            qt[:, t * IT + olo : (t + 1) * IT],
                            start=True,
                            stop=True,
                        )
                        pr = prp.tile([P, 2 * IT], dt.bfloat16, name="pr")
                        nc.scalar.activation(
                            pr[:, ohi : 2 * IT], sp[:, ohi : 2 * IT],
                            AF.Exp, scale=SCALE,
                        )
                        # both 64-wide edge blocks ([ohi,+64) and [IT+olo,+64),
                        # stride EW apart) in one strided op; X shifts the
                        # window so the 2*EW base slice stays inside the tile
                        X = max(0, ohi - (2 * IT - 2 * EW))
                        eap = pr[:, ohi - X : ohi - X + 2 * EW].rearrange(
                            "p (u v) -> p u v", u=2
                        )[:, :, X : X + 64]
                        ge.tensor_mul(eap, eap, edge2)
                        defer((pr, j_hi, j_lo, ohi, olo))
                    consume_pair(state["pend"], last=True)

                    def fin(hq=hq, t=t, yps=yps, qds=tuple(qds)):
                        dps = pB.tile([P, IT], dt.float32, name="dps",
                                      tag="dps", bufs=1)
                        for i, (qd, lo) in enumerate(qds):
                            nc.tensor.matmul(
                                dps[:, lo:IT], onesm[:], qd[:, lo:IT],
                                start=(i == 0), stop=(i == len(qds) - 1),
                            )
                        rec = recp.tile([P, IT], dt.float32, name="rec")
                        nc.vector.reciprocal_approx_fast(rec[:], dps[:])
                        nc.vector.tensor_mul(
                            yT[:, hq * S_LOC + t * IT : hq * S_LOC + (t + 1) * IT],
                            yps[:],
                            rec[:],
                        )

                    pending[0] = fin

            # software pipeline: head hq's attention is flanked by the
            # (hq+2) q-proj t1-group + rope and the (hq+3) t0-group, giving
            # each deferred finalize ~7us of independent PE work to hide
            # the trailing exp->edge->reduce chain of the previous head.
            q_part(2, 0, pB, "qps")
            for hq in range(NH):
                attention(hq)
                if hq + 2 < NH:
                    q_part(hq + 2, 1, pB, "qps")
                    qtiles[hq + 2] = rope_finish(hq + 2)
                if hq + 3 < NH:
                    q_part(hq + 3, 0, pB, "qps")
            flush_pending()

        s_ab.close()  # free xt / kT / vv / q pools before the projection phase

        # ---------------- phase C: out projection + rmsnorm -------------
        with ExitStack() as phC:
            wpbp = phC.enter_context(tc.tile_pool(name="wpbp", bufs=1))
            wpc = wpbp.tile([P, (HT - WPA) * H], dt.bfloat16)
            for i in range(HT - WPA):
                nc.sync.dma_start(
                    wpc[:, i * H : (i + 1) * H], wp_d.ap()[WPA + i]
                )
            nwp = phC.enter_context(tc.tile_pool(name="nwp", bufs=1))
            nw = nwp.tile([P, H], dt.float32)
            nc.sync.dma_start(nw[:], nw_d.ap())

            outp = phC.enter_context(tc.tile_pool(name="outp", bufs=2))
            sqp = phC.enter_context(tc.tile_pool(name="sqp", bufs=2))
            smp = phC.enter_context(tc.tile_pool(name="smp", bufs=2))
            po = phC.enter_context(tc.tile_pool(name="po", bufs=8, space="PSUM"))

            mult = mybir.AluOpType.mult

            nslice = S_LOC // P
            for sl in range(nslice):
                # last slice runs o-outer so the norm chain pipelines with
                # the matmuls and the tail after the final matmul is short
                o_outer = sl == nslice - 1
                pso = [po.tile([P, OT], dt.float32, name="pso") for _ in range(NO)]
                ot = outp.tile([P, H], dt.float32, name="ot")
                ssqs = []

                def chunk_post(o):
                    sq = sqp.tile([P, OT], dt.float32, name="sq")
                    sso = smp.tile([P, 1], dt.float32, name="sso", tag="sso", bufs=8)
                    nc.scalar.activation(
                        sq[:], pso[o][:], AF.Square, accum_out=sso[:]
                    )
                    nc.scalar.activation(
                        ot[:, o * OT : (o + 1) * OT], pso[o][:], AF.Copy
                    )
                    ssqs.append(sso)

                lhss = [
                    yT[:, h * S_LOC + sl * P : h * S_LOC + (sl + 1) * P]
                    for h in range(HT)
                ]
                def wslice(h, o):
                    if h < WPA:
                        return wpa[:, h * H + o * OT : h * H + (o + 1) * OT]
                    hh = h - WPA
                    return wpc[:, hh * H + o * OT : hh * H + (o + 1) * OT]

                if o_outer:
                    for o in range(NO):
                        for h in range(HT):
                            nc.tensor.matmul(
                                pso[o][:],
                                lhss[h],
                                wslice(h, o),
                                start=(h == 0),
                                stop=(h == HT - 1),
                            )
                        chunk_post(o)
                else:
                    for h in range(HT):
                        for o in range(NO):
                            nc.tensor.matmul(
                                pso[o][:],
                                lhss[h],
                                wslice(h, o),
                                start=(h == 0),
                                stop=(h == HT - 1),
                            )
                    for o in range(NO):
                        chunk_post(o)
                sa = smp.tile([P, 1], dt.float32, name="sa")
                sb = smp.tile([P, 1], dt.float32, name="sb")
                nc.vector.tensor_add(sa[:], ssqs[0][:], ssqs[1][:])
                nc.vector.tensor_add(sb[:], ssqs[2][:], ssqs[3][:])
                ssq = smp.tile([P, 1], dt.float32, name="ssq")
                nc.vector.tensor_add(ssq[:], sa[:], sb[:])
                rms = smp.tile([P, 1], dt.float32, name="rms")
                nc.scalar.activation(
                    rms[:], ssq[:], AF.Sqrt, bias=epsb[:], scale=1.0 / H
                )
                rr = smp.tile([P, 1], dt.float32, name="rr")
                nc.vector.reciprocal(rr[:], rms[:])
                for half in range(2):
                    for o in (2 * half, 2 * half + 1):
                        nc.vector.scalar_tensor_tensor(
                            ot[:, o * OT : (o + 1) * OT],
                            ot[:, o * OT : (o + 1) * OT],
                            rr[:],
                            nw[:, o * OT : (o + 1) * OT],
                            mult,
                            mult,
                        )
                    nc.sync.dma_start(
                        out_d.ap()[sl * P : (sl + 1) * P, half * H // 2 : (half + 1) * H // 2],
                        ot[:, half * H // 2 : (half + 1) * H // 2],
                    )

    nc.compile()
    return nc


# ---------------------------------------------------------------- host side
def _host_shared(w_attn, w_proj, norm_w):
    """Core-independent packed tensors."""
    f32 = np.float32

    def perm_halves(w):  # [H, n, HD] even/odd pairs -> halves
        return np.concatenate([w[..., 0::2], w[..., 1::2]], axis=-1)

    wq = perm_halves(w_attn[:, :H].reshape(H, NH, HD))
    wq = np.ascontiguousarray(
        wq.reshape(HT, P, NH, HD).transpose(2, 1, 0, 3)
    ).astype(BF16)
    wk = perm_halves(w_attn[:, H : H + KVC].reshape(H, NKV, HD))
    wk = np.ascontiguousarray(
        wk.reshape(HT, P, NKV, HD).transpose(2, 1, 0, 3)
    ).astype(BF16)
    wv = np.ascontiguousarray(
        w_attn[:, H + KVC :].reshape(HT, P, KVC)
    ).astype(BF16)
    wp = np.ascontiguousarray(w_proj.reshape(HT, P, H)).astype(BF16)

    p, f = np.meshgrid(np.arange(P), np.arange(64), indexing="ij")
    # parity 0: query col f (global row 2f within the 64-block diagonal)
    edge0 = (2 * f >= p).astype(BF16)
    # parity 1: query 2f+1 vs key (p^1) (pair-swapped x columns)
    edge1 = (2 * f + 1 >= (p ^ 1)).astype(BF16)
    # duplicated side by side: one strided op masks both blocks of a pair
    edge0 = np.ascontiguousarray(np.concatenate([edge0, edge0], axis=1))
    edge1 = np.ascontiguousarray(np.concatenate([edge1, edge1], axis=1))

    nw = np.ascontiguousarray(
        np.broadcast_to(norm_w.astype(f32), (P, H))
    )
    return wq, wk, wv, wp, (edge0, edge1), nw


def _cos_sin(pos):
    f32 = np.float32
    inv = 1.0 / (
        10000.0 ** (np.arange(0, HD, 2, dtype=f32) / f32(HD))
    )
    ang = inv[:, None].astype(f32) * pos[None, :].astype(f32)  # [HH, N]
    c, s = np.cos(ang).astype(BF16), np.sin(ang).astype(BF16)
    # duplicated across both partition halves (walrus wants equal base
    # partitions for SBUF tensor-tensor inputs)
    return (
        np.ascontiguousarray(np.concatenate([c, c], axis=0)),
        np.ascontiguousarray(np.concatenate([s, s], axis=0)),
    )


def make_in_maps(x, w_attn, w_proj, norm_w):
    x = np.asarray(x, dtype=np.float32)
    w_attn = np.asarray(w_attn, dtype=np.float32)
    w_proj = np.asarray(w_proj, dtype=np.float32)
    norm_w = np.asarray(norm_w, dtype=np.float32)

    wq, wk, wv, wp, (edge0, edge1), nw = _host_shared(w_attn, w_proj, norm_w)

    kc0, ks0 = _cos_sin(np.arange(S, dtype=np.float32))          # parity 0
    # parity 1: column j holds global row j^1 (pair-swapped x columns)
    kc1, ks1 = _cos_sin((np.arange(S) ^ 1).astype(np.float32))
    qc0, qs0 = _cos_sin(2.0 * np.arange(S_LOC, dtype=np.float32))
    qc1, qs1 = _cos_sin(2.0 * np.arange(S_LOC, dtype=np.float32) + 1.0)

    in_maps = []
    for c in range(N_CORES):
        b, par = c // 2, c % 2
        xt = x[b].T.astype(BF16)
        if par:
            xt = xt[:, np.arange(S) ^ 1]  # swap adjacent column pairs
        # parity-packed contiguous copy for the q projection
        xq = np.ascontiguousarray(
            xt[:, 0::2].reshape(HT, P, S_LOC)
        )
        xt = np.ascontiguousarray(xt.reshape(HT, P, S))
        in_maps.append(
            {
                "xt": xt,
                "xq": xq,
                "wq": wq,
                "wk": wk,
                "wv": wv,
                "wp": wp,
                "qcos": qc1 if par else qc0,
                "qsin": qs1 if par else qs0,
                "kcos": kc1 if par else kc0,
                "ksin": ks1 if par else ks0,
                "edge": edge1 if par else edge0,
                "nw": nw,
            }
        )
    return in_maps


def assemble_out(results):
    out = np.empty((B, S, H), dtype=np.float32)
    for c in range(N_CORES):
        b, par = c // 2, c % 2
        out[b, par::2, :] = results[c]["out"]
    return out


def kernel(x, w_attn, w_proj, norm_w):
    from concourse import bass_utils

    if "nc" not in _CACHE:
        _CACHE["nc"] = _build_nc()
    nc = _CACHE["nc"]

    in_maps = make_in_maps(x, w_attn, w_proj, norm_w)
    res = bass_utils.run_bass_kernel_spmd(
        nc, in_maps, core_ids=list(range(N_CORES))
    )
    return assemble_out(res.results)
